# revision 13
# baseline (speedup 1.0000x reference)
"""Bass/Tile kernel v4 for nn_Head: wide transposed AV + host-side softmax
normalization.

Sharding (as v3): core (b, h) = (core//2, core%2); local block t -> global
row-block T = 2t + h. h-dependence lives in the two mask inputs.

Changes vs v3:
- AV computed transposed: out_T[d, i] = sum_c v_c^T @ E_c[:, span]; one wide
  matmul per (slice, chunk) plus one 128-col masked matmul, instead of up to
  4 narrow 128-col matmuls. ~2.5x fewer AV matmuls, larger moving dims.
- v built from vT via DMA XBAR transpose (no PE transposes, no DVE copies).
- Softmax denominator + division moved to host: kernel ships the raw
  numerator partials (numT, [d,i] f32) and exp-sum tiles (acc, f16).
- AV(0)/AV(1) braided into the projection/ST phase; AV(2)/AV(3) tail.
- No identity matrices / PE transposes at all.

Mask data (per h): for sub-block u of slice s, chunk c0=8s+2u gets maskA,
c0+1 gets maskB:
  h=0 (T=c0):   maskA = strict-lower, maskB = ones
  h=1 (T=c0+1): maskA = zeros,        maskB = strict-lower
Wide-AV form: for (s, c), rel = c-8s: full-width matmul covers columns
[0, clamp(rel//2,0,4)*128); masked 128-col matmul at u_m = rel//2 (if < 4)
with maskA if rel even else maskB.
"""
import sys
sys.path.insert(0, '/opt/trn_rl_repo')

from contextlib import ExitStack

import numpy as np
import ml_dtypes

import concourse.bass as bass
import concourse.bacc as bacc
import concourse.tile as tile
from concourse import mybir
from concourse.bass_utils import run_bass_kernel_spmd

F32 = mybir.dt.float32
BF16 = mybir.dt.bfloat16
F16 = mybir.dt.float16
NPBF16 = ml_dtypes.bfloat16
NPF16 = np.float16

B, N, M, C, D = 4, 4096, 4096, 1024, 128
NCORES = 8
NLOC = N // 2              # 2048 rows per core
CCH = C // 128             # 8 contraction chunks
JCH = M // 128             # 32 j-chunks
NSL = 4                    # i-slices of 512
SCALE = 1.0 / np.sqrt(np.float32(D))

_CACHE = {}
TRACE = False


def build_nc():
    nc = bacc.Bacc("TRN2", target_bir_lowering=False, debug=False,
                   num_devices=NCORES)
    xsT_d = nc.dram_tensor("xsT", [C, NLOC], BF16, kind="ExternalInput").ap()
    yT_d = nc.dram_tensor("yT", [C, M], BF16, kind="ExternalInput").ap()
    wq_d = nc.dram_tensor("wq", [C, D], BF16, kind="ExternalInput").ap()
    wk_d = nc.dram_tensor("wk", [C, D], BF16, kind="ExternalInput").ap()
    wv_d = nc.dram_tensor("wv", [C, D], BF16, kind="ExternalInput").ap()
    maskAB_d = nc.dram_tensor("maskAB", [128, 256], F16, kind="ExternalInput").ap()
    numT_d = nc.dram_tensor("numT", [NSL, 128, 512], F32, kind="ExternalOutput").ap()
    acc_d = nc.dram_tensor("accout", [NSL, 128, 1024], F16, kind="ExternalOutput").ap()

    with tile.TileContext(nc) as tc:
        with ExitStack() as ctx:
            const = ctx.enter_context(tc.tile_pool(name="const", bufs=1))
            stgx = ctx.enter_context(tc.tile_pool(name="stgx", bufs=1))
            stgy = ctx.enter_context(tc.tile_pool(name="stgy", bufs=2))
            stgv = ctx.enter_context(tc.tile_pool(name="stgv", bufs=2))
            big = ctx.enter_context(tc.tile_pool(name="big", bufs=1))
            etp = ctx.enter_context(tc.tile_pool(name="etp", bufs=26))
            accp = ctx.enter_context(tc.tile_pool(name="accp", bufs=4))
            outp = ctx.enter_context(tc.tile_pool(name="outp", bufs=2))
            # PSUM 8 banks: shared proj/ST pool 3x2 + AV 2x1
            stps = ctx.enter_context(tc.tile_pool(name="stps", bufs=3, space="PSUM"))
            avps = ctx.enter_context(tc.tile_pool(name="avps", bufs=2, space="PSUM"))

            # ---- constants: weights on the scalar (ACT) hwdge queue so the
            # sync queue is free for the yT group-0 stream ----
            wq_sb = const.tile([128, CCH, D], BF16)
            wk_sb = const.tile([128, CCH, D], BF16)
            wv_sb = const.tile([128, CCH, D], BF16)
            nc.scalar.dma_start(out=wk_sb, in_=wk_d.rearrange("(c p) d -> p c d", p=128))
            nc.scalar.dma_start(out=wq_sb, in_=wq_d.rearrange("(c p) d -> p c d", p=128))
            nc.scalar.dma_start(out=wv_sb, in_=wv_d.rearrange("(c p) d -> p c d", p=128))
            maskAB_sb = const.tile([128, 2, 128], F16)
            nc.sync.dma_start(out=maskAB_sb,
                              in_=maskAB_d.rearrange("p (a b) -> p a b", a=2))

            # ---- residents ----
            qT_sb = big.tile([128, NLOC], BF16)       # q^T [d, i]
            kT_sb = big.tile([128, M], BF16)          # k^T [d, j]
            v_sb = big.tile([128, JCH, D], BF16)      # v [j-in-chunk, chunk, d]

            xsT_view = xsT_d.rearrange("(c p) n -> p c n", p=128)
            yT_view = yT_d.rearrange("(c p) m -> p c m", p=128)

            def drain(dst, src_ap):
                # all drains on DVE: scalar-engine drains head-of-line block
                # the exp FIFO (ACT is strict FIFO)
                nc.vector.tensor_copy(dst, src_ap)

            ET_tiles = {}     # (slice, cc) -> tile [128, 2, 512] f16
            acc_tiles = {}
            av_tiles = {}

            def get_et(s, c):
                """AP of the exp tile for (slice s, chunk c): [128, 512]."""
                return ET_tiles[(s, c // 2)][:, c % 2, :]

            def emit_st_tile(s, cc):
                """ST matmuls + exp + denominator add for chunks 2cc,2cc+1."""
                qs = qT_sb[:, s * 512:(s + 1) * 512]
                st_ps = stps.tile([128, 2, 512], F32, tag="st")
                for half in range(2):
                    c = 2 * cc + half
                    nc.tensor.matmul(st_ps[:, half, :],
                                     kT_sb[:, c * 128:(c + 1) * 128], qs,
                                     start=True, stop=True)
                # slice 0's chunks 0,1 are AV-consumed, so its acc cannot
                # alias the first exp tile (later adds would corrupt them).
                first = (cc == 0 and s > 0)
                if first:
                    ET = acc_tiles[s]
                else:
                    ET = etp.tile([128, 2, 512], F16, tag="ET")
                nc.scalar.activation(ET.rearrange("p a b -> p (a b)"),
                                     st_ps.rearrange("p a b -> p (a b)"),
                                     mybir.ActivationFunctionType.Exp,
                                     scale=float(SCALE))
                ET_tiles[(s, cc)] = ET
                if not first:
                    acc = acc_tiles[s]
                    if cc == 0:
                        nc.vector.tensor_copy(acc, ET)
                    else:
                        nc.vector.tensor_add(
                            acc.rearrange("p a b -> p (a b)"),
                            acc.rearrange("p a b -> p (a b)"),
                            ET.rearrange("p a b -> p (a b)"))
                # mask the diagonal block IN PLACE (the denominator add
                # above already consumed the raw values, and the masked block
                # is column-adjacent to the unmasked span, so AV needs only
                # one wide matmul per chunk)
                for half in range(2):
                    c = 2 * cc + half
                    rel = c - 8 * s
                    if 0 <= rel <= 7:
                        um = rel // 2
                        par = rel % 2
                        blk = ET[:, half, um * 128:(um + 1) * 128]
                        nc.vector.tensor_mul(blk, blk, maskAB_sb[:, par, :])

            def emit_av_chunk(s, c):
                """One wide AV matmul consuming exp of chunk c for slice s.
                Width covers the unmasked span plus the in-place-masked
                diagonal block (column-adjacent)."""
                rel = c - 8 * s
                if rel < 0:
                    return
                av_ps = av_tiles[s]
                w = min(rel // 2 + 1, 4) * 128
                nc.tensor.matmul(av_ps[:, 0:w],
                                 v_sb[:, c, :], get_et(s, c)[:, 0:w],
                                 start=(rel == 0), stop=(c == JCH - 1),
                                 skip_group_check=True)

            def emit_out(s):
                av_ps = av_tiles.pop(s)
                o_sb = outp.tile([128, 512], F32, tag="o")
                nc.vector.tensor_copy(o_sb, av_ps)
                nc.sync.dma_start(out=numT_d[s], in_=o_sb)

            def emit_acc_out(s):
                acc = acc_tiles[s]
                nc.sync.dma_start(out=acc_d[s],
                                  in_=acc.rearrange("p a b -> p (a b)"))

            # ---- x DMA: slices 0,1 on scalar queue, slices 2,3 on gpsimd,
            # keeping the sync queue clear for the yT stream ----
            x_sb = stgx.tile([128, CCH, NLOC], BF16)

            def emit_x_dma(it):
                for c in range(CCH):
                    nc.scalar.dma_start(
                        out=x_sb[:, c, it * 1024:(it + 1) * 1024],
                        in_=xsT_view[:, c, it * 1024:(it + 1) * 1024])

            def proj_half(w_sb, yT, half):
                p_ps = stps.tile([128, 512], F32, tag="st")
                for c in range(CCH):
                    nc.tensor.matmul(p_ps, w_sb[:, c, :],
                                     yT[:, c, half * 512:(half + 1) * 512],
                                     start=(c == 0), stop=(c == CCH - 1))
                return p_ps

            def q_half(sl):
                p_ps = stps.tile([128, 512], F32, tag="st")
                for c in range(CCH):
                    nc.tensor.matmul(
                        p_ps, wq_sb[:, c, :],
                        x_sb[:, c, sl * 512:(sl + 1) * 512],
                        start=(c == 0), stop=(c == CCH - 1))
                drain(qT_sb[:, sl * 512:(sl + 1) * 512], p_ps)

            for s in range(NSL):
                acc_tiles[s] = accp.tile([128, 2, 512], F16, tag="acc",
                                         name=f"acc{s}")
            av_tiles[0] = avps.tile([128, 512], F32, tag="av", name="av0")
            av_tiles[1] = avps.tile([128, 512], F32, tag="av", name="av1")

            # ST catch-up schedules for slices 2,3 (cc lists per group),
            # spread to keep ACT (exp) load even across groups
            s2_cc = {0: [0, 1], 1: [2, 3, 4, 5], 2: [6, 7, 8, 9, 10],
                     3: [11, 12, 13, 14, 15]}
            s3_cc = {1: [0, 1, 2, 3, 4], 2: [5, 6, 7, 8, 9, 10],
                     3: [11, 12, 13, 14, 15]}

            # ---- main pipeline over 4 j-groups of 1024 (8 chunks each);
            # yT DMA issued one group ahead so transfers hide under compute
            # and the group-jt transposes never delay group jt+1's stream ----
            yT_tiles = {}

            def issue_yt(jt):
                t = stgy.tile([128, CCH, 1024], BF16, tag="yT")
                for c in range(CCH):
                    nc.sync.dma_start(out=t[:, c, :],
                                      in_=yT_view[:, c, jt * 1024:(jt + 1) * 1024])
                yT_tiles[jt] = t

            issue_yt(0)
            for jt in range(M // 1024):
                if jt + 1 < M // 1024:
                    issue_yt(jt + 1)
                yT = yT_tiles.pop(jt)
                if jt == 0:
                    emit_x_dma(0)
                    emit_x_dma(1)

                # kT halves + ST slices 0,1 + braided AV(0,1)
                for half in range(2):
                    p = proj_half(wk_sb, yT, half)
                    off = jt * 1024 + half * 512
                    drain(kT_sb[:, off:off + 512], p)
                    if jt == 0 and half == 0:
                        q_half(0)
                        q_half(1)
                    # AV braid between proj and ST: these matmuls depend only
                    # on last group's exps, filling the kT-drain latency
                    if jt > 0:
                        base = 8 * (jt - 1) + 4 * half
                        for c in range(base, base + 4):
                            emit_av_chunk(0, c)
                            emit_av_chunk(1, c)
                    for cc in (4 * jt + 2 * half, 4 * jt + 2 * half + 1):
                        emit_st_tile(0, cc)
                        emit_st_tile(1, cc)

                # vT halves -> v via DMA XBAR transpose + catch-up STs
                vstage = stgv.tile([128, 1024], BF16, tag="vT")
                for half in range(2):
                    p = proj_half(wv_sb, yT, half)
                    drain(vstage[:, half * 512:(half + 1) * 512], p)
                    if jt == 0 and half == 0:
                        q_half(2)
                    if jt == 0 and half == 1:
                        q_half(3)
                    if half == 0:
                        for cc in s2_cc.get(jt, []):
                            emit_st_tile(2, cc)
                    else:
                        for cc in s3_cc.get(jt, []):
                            emit_st_tile(3, cc)
                    nc.sync.dma_start_transpose(
                        v_sb[:, 8 * jt + 4 * half:8 * jt + 4 * half + 4, :],
                        vstage[:, half * 512:(half + 1) * 512])

            # ---- tail: last group's AV(0,1), then AV(2), AV(3) ----
            for c in range(24, 32):
                emit_av_chunk(0, c)
                emit_av_chunk(1, c)
            emit_acc_out(0)
            emit_acc_out(1)
            emit_out(0)
            emit_out(1)

            av_tiles[2] = stps.tile([128, 512], F32, tag="st", name="av2")
            av_tiles[3] = stps.tile([128, 512], F32, tag="st", name="av3")
            emit_acc_out(2)
            emit_acc_out(3)
            for c in range(16, 32):
                emit_av_chunk(2, c)
                if c >= 24:
                    emit_av_chunk(3, c)
            emit_out(2)
            emit_out(3)

    nc.compile()
    return nc


def _get_nc():
    if "nc" not in _CACHE:
        _CACHE["nc"] = build_nc()
    return _CACHE["nc"]


def _make_masks(h):
    m = np.zeros((128, 256), dtype=np.float32)
    sl = np.tril(np.ones((128, 128), dtype=np.float32), k=-1)
    if h == 0:
        m[:, 0:128] = sl
        m[:, 128:256] = 1.0
    else:
        m[:, 0:128] = 0.0
        m[:, 128:256] = sl
    return m.astype(NPF16)


def kernel(x, y, Wq, Wk, Wv):
    nc = _get_nc()
    xb = x.astype(NPBF16)
    yb = y.astype(NPBF16)
    wqb = Wq.astype(NPBF16)
    wkb = Wk.astype(NPBF16)
    wvb = Wv.astype(NPBF16)

    in_maps = []
    yT = {b: np.ascontiguousarray(yb[b].T) for b in range(B)}
    for core in range(NCORES):
        b, h = divmod(core, 2)
        xs = xb[b].reshape(N // 128, 128, C)[h::2].reshape(NLOC, C)
        in_maps.append({
            "xsT": np.ascontiguousarray(xs.T),
            "yT": yT[b],
            "wq": wqb, "wk": wkb, "wv": wvb,
            "maskAB": _make_masks(h),
        })

    if TRACE:
        import tempfile
        tdir = tempfile.mkdtemp(prefix="attn_trace_")
        _CACHE["trace_dir"] = tdir
        res = run_bass_kernel_spmd(nc, in_maps, list(range(NCORES)),
                                   trace=True, tmpdir=tdir)
        _CACHE["exec_time_ns"] = res.exec_time_ns
    else:
        res = run_bass_kernel_spmd(nc, in_maps, list(range(NCORES)))

    out = np.empty((B, N, D), dtype=np.float32)
    for core in range(NCORES):
        b, h = divmod(core, 2)
        num = res.results[core]["numT"]                       # [4,128,512]
        accv = res.results[core]["accout"].astype(np.float32)  # [4,128,1024]
        for s in range(NSL):
            Z = accv[s].reshape(128, 2, 512).sum(axis=(0, 1))  # [512]
            o = (num[s] / Z[None, :]).T.reshape(4, 128, D)     # [u, r, d]
            for u in range(4):
                T = 2 * (4 * s + u) + h
                out[b, T * 128:(T + 1) * 128] = o[u]
    return out


# revision 15
# speedup vs baseline: 1.0762x; 1.0762x over previous
"""Bass/Tile kernel v4 for nn_Head: wide transposed AV + host-side softmax
normalization.

Sharding (as v3): core (b, h) = (core//2, core%2); local block t -> global
row-block T = 2t + h. h-dependence lives in the two mask inputs.

Changes vs v3:
- AV computed transposed: out_T[d, i] = sum_c v_c^T @ E_c[:, span]; one wide
  matmul per (slice, chunk) plus one 128-col masked matmul, instead of up to
  4 narrow 128-col matmuls. ~2.5x fewer AV matmuls, larger moving dims.
- v built from vT via DMA XBAR transpose (no PE transposes, no DVE copies).
- Softmax denominator + division moved to host: kernel ships the raw
  numerator partials (numT, [d,i] f32) and exp-sum tiles (acc, f16).
- AV(0)/AV(1) braided into the projection/ST phase; AV(2)/AV(3) tail.
- No identity matrices / PE transposes at all.

Mask data (per h): for sub-block u of slice s, chunk c0=8s+2u gets maskA,
c0+1 gets maskB:
  h=0 (T=c0):   maskA = strict-lower, maskB = ones
  h=1 (T=c0+1): maskA = zeros,        maskB = strict-lower
Wide-AV form: for (s, c), rel = c-8s: full-width matmul covers columns
[0, clamp(rel//2,0,4)*128); masked 128-col matmul at u_m = rel//2 (if < 4)
with maskA if rel even else maskB.
"""
import sys
sys.path.insert(0, '/opt/trn_rl_repo')

from contextlib import ExitStack

import numpy as np
import ml_dtypes

import concourse.bass as bass
import concourse.bacc as bacc
import concourse.tile as tile
from concourse import mybir
from concourse.bass_utils import run_bass_kernel_spmd

F32 = mybir.dt.float32
BF16 = mybir.dt.bfloat16
F16 = mybir.dt.float16
NPBF16 = ml_dtypes.bfloat16
NPF16 = np.float16

B, N, M, C, D = 4, 4096, 4096, 1024, 128
NCORES = 8
NLOC = N // 2              # 2048 rows per core
CCH = C // 128             # 8 contraction chunks
JCH = M // 128             # 32 j-chunks
NSL = 4                    # i-slices of 512
SCALE = 1.0 / np.sqrt(np.float32(D))

_CACHE = {}
TRACE = False


def build_nc():
    nc = bacc.Bacc("TRN2", target_bir_lowering=False, debug=False,
                   num_devices=NCORES)
    xsT_d = nc.dram_tensor("xsT", [C, NLOC], BF16, kind="ExternalInput").ap()
    yT_d = nc.dram_tensor("yT", [C, M], BF16, kind="ExternalInput").ap()
    wq_d = nc.dram_tensor("wq", [C, D], BF16, kind="ExternalInput").ap()
    wk_d = nc.dram_tensor("wk", [C, D], BF16, kind="ExternalInput").ap()
    wv_d = nc.dram_tensor("wv", [C, D], BF16, kind="ExternalInput").ap()
    maskAB_d = nc.dram_tensor("maskAB", [128, 256], F16, kind="ExternalInput").ap()
    numT_d = nc.dram_tensor("numT", [NSL, 128, 512], F32, kind="ExternalOutput").ap()
    acc_d = nc.dram_tensor("accout", [NSL, 128, 1024], F16, kind="ExternalOutput").ap()

    with tile.TileContext(nc) as tc:
        with ExitStack() as ctx:
            const = ctx.enter_context(tc.tile_pool(name="const", bufs=1))
            stgx = ctx.enter_context(tc.tile_pool(name="stgx", bufs=1))
            stgy = ctx.enter_context(tc.tile_pool(name="stgy", bufs=2))
            stgv = ctx.enter_context(tc.tile_pool(name="stgv", bufs=2))
            big = ctx.enter_context(tc.tile_pool(name="big", bufs=1))
            accp = ctx.enter_context(tc.tile_pool(name="accp", bufs=4))
            outp = ctx.enter_context(tc.tile_pool(name="outp", bufs=2))
            # PSUM 8 banks: shared proj/ST pool 3x2 + AV 2x1
            stps = ctx.enter_context(tc.tile_pool(name="stps", bufs=3, space="PSUM"))
            avps = ctx.enter_context(tc.tile_pool(name="avps", bufs=2, space="PSUM"))

            # ---- constants: weights on the scalar (ACT) hwdge queue so the
            # sync queue is free for the yT group-0 stream ----
            wq_sb = const.tile([128, CCH, D], BF16)
            wk_sb = const.tile([128, CCH, D], BF16)
            wv_sb = const.tile([128, CCH, D], BF16)
            nc.scalar.dma_start(out=wk_sb, in_=wk_d.rearrange("(c p) d -> p c d", p=128))
            nc.scalar.dma_start(out=wq_sb, in_=wq_d.rearrange("(c p) d -> p c d", p=128))
            nc.scalar.dma_start(out=wv_sb, in_=wv_d.rearrange("(c p) d -> p c d", p=128))
            maskAB_sb = const.tile([128, 2, 128], F16)
            nc.sync.dma_start(out=maskAB_sb,
                              in_=maskAB_d.rearrange("p (a b) -> p a b", a=2))

            # ---- residents ----
            qT_sb = big.tile([128, NLOC], BF16)       # q^T [d, i]
            kT_sb = big.tile([128, M], BF16)          # k^T [d, j]
            v_sb = big.tile([128, JCH, D], BF16)      # v [j-in-chunk, chunk, d]
            # exp tiles live in one resident tensor with manual slot
            # rotation (sub-range dep tracking handles reuse); a 26-buf
            # pool would add ~NETS teardown semaphore rounds (~4us).
            # NETS must exceed the reuse distance to every saved tile: the
            # earliest saved allocation (s2 cc8, ~idx 33 of 60) must not be
            # overwritten before the tail AV(2) reads it.
            NETS = 34
            ET_all = big.tile([128, NETS, 2, 512], F16)
            et_ctr = [0]

            xsT_view = xsT_d.rearrange("(c p) n -> p c n", p=128)
            yT_view = yT_d.rearrange("(c p) m -> p c m", p=128)

            def drain(dst, src_ap):
                # all drains on DVE: scalar-engine drains head-of-line block
                # the exp FIFO (ACT is strict FIFO)
                nc.vector.tensor_copy(dst, src_ap)

            ET_tiles = {}     # (slice, cc) -> tile [128, 2, 512] f16
            acc_tiles = {}
            av_tiles = {}

            def get_et(s, c):
                """AP of the exp tile for (slice s, chunk c): [128, 512]."""
                return ET_tiles[(s, c // 2)][:, c % 2, :]

            def emit_st_tile(s, cc):
                """ST matmuls + exp + denominator add for chunks 2cc,2cc+1."""
                qs = qT_sb[:, s * 512:(s + 1) * 512]
                st_ps = stps.tile([128, 2, 512], F32, tag="st")
                for half in range(2):
                    c = 2 * cc + half
                    nc.tensor.matmul(st_ps[:, half, :],
                                     kT_sb[:, c * 128:(c + 1) * 128], qs,
                                     start=True, stop=True)
                # slice 0's chunks 0,1 are AV-consumed, so its acc cannot
                # alias the first exp tile (later adds would corrupt them).
                first = (cc == 0 and s > 0)
                if first:
                    ET = acc_tiles[s]
                else:
                    ET = ET_all[:, et_ctr[0] % NETS]
                    et_ctr[0] += 1
                nc.scalar.activation(ET.rearrange("p a b -> p (a b)"),
                                     st_ps.rearrange("p a b -> p (a b)"),
                                     mybir.ActivationFunctionType.Exp,
                                     scale=float(SCALE))
                ET_tiles[(s, cc)] = ET
                if not first:
                    acc = acc_tiles[s]
                    if cc == 0:
                        nc.vector.tensor_copy(acc, ET)
                    else:
                        nc.vector.tensor_add(
                            acc.rearrange("p a b -> p (a b)"),
                            acc.rearrange("p a b -> p (a b)"),
                            ET.rearrange("p a b -> p (a b)"))
                # mask the diagonal block IN PLACE (the denominator add
                # above already consumed the raw values, and the masked block
                # is column-adjacent to the unmasked span, so AV needs only
                # one wide matmul per chunk)
                for half in range(2):
                    c = 2 * cc + half
                    rel = c - 8 * s
                    if 0 <= rel <= 7:
                        um = rel // 2
                        par = rel % 2
                        blk = ET[:, half, um * 128:(um + 1) * 128]
                        nc.vector.tensor_mul(blk, blk, maskAB_sb[:, par, :])

            def emit_av_chunk(s, c):
                """One wide AV matmul consuming exp of chunk c for slice s.
                Width covers the unmasked span plus the in-place-masked
                diagonal block (column-adjacent)."""
                rel = c - 8 * s
                if rel < 0:
                    return
                av_ps = av_tiles[s]
                w = min(rel // 2 + 1, 4) * 128
                nc.tensor.matmul(av_ps[:, 0:w],
                                 v_sb[:, c, :], get_et(s, c)[:, 0:w],
                                 start=(rel == 0), stop=(c == JCH - 1),
                                 skip_group_check=True)

            def emit_out(s):
                av_ps = av_tiles.pop(s)
                o_sb = outp.tile([128, 512], F32, tag="o")
                nc.vector.tensor_copy(o_sb, av_ps)
                nc.sync.dma_start(out=numT_d[s], in_=o_sb)

            def emit_acc_out(s):
                acc = acc_tiles[s]
                nc.sync.dma_start(out=acc_d[s],
                                  in_=acc.rearrange("p a b -> p (a b)"))

            # ---- x DMA, chunk-major so q-proj matmuls pipeline with
            # arrivals: slices 0,1 on the scalar hwdge queue (pre-exp),
            # slices 2,3 on sync after the yT(1) prefetch ----
            x_sb = stgx.tile([128, CCH, NLOC], BF16)

            def emit_x_dma(it):
                eng = nc.scalar if it == 0 else nc.sync
                for c in range(CCH):
                    eng.dma_start(
                        out=x_sb[:, c, it * 1024:(it + 1) * 1024],
                        in_=xsT_view[:, c, it * 1024:(it + 1) * 1024])

            def proj_half(w_sb, yT, half):
                p_ps = stps.tile([128, 512], F32, tag="st")
                for c in range(CCH):
                    nc.tensor.matmul(p_ps, w_sb[:, c, :],
                                     yT[:, c, half * 512:(half + 1) * 512],
                                     start=(c == 0), stop=(c == CCH - 1))
                return p_ps

            def q_half(sl):
                p_ps = stps.tile([128, 512], F32, tag="st")
                for c in range(CCH):
                    nc.tensor.matmul(
                        p_ps, wq_sb[:, c, :],
                        x_sb[:, c, sl * 512:(sl + 1) * 512],
                        start=(c == 0), stop=(c == CCH - 1))
                drain(qT_sb[:, sl * 512:(sl + 1) * 512], p_ps)

            for s in range(NSL):
                acc_tiles[s] = accp.tile([128, 2, 512], F16, tag="acc",
                                         name=f"acc{s}")
            av_tiles[0] = avps.tile([128, 512], F32, tag="av", name="av0")
            av_tiles[1] = avps.tile([128, 512], F32, tag="av", name="av1")

            # ST catch-up schedules for slices 2,3 (cc lists per group),
            # spread to keep ACT (exp) load even across groups
            s2_cc = {1: [0, 1, 2, 3, 4, 5], 2: [6, 7, 8, 9, 10],
                     3: [11, 12, 13, 14, 15]}
            s3_cc = {1: [0, 1, 2, 3], 2: [4, 5, 6, 7, 8, 9],
                     3: [10, 11, 12, 13, 14, 15]}

            # ---- main pipeline over 4 j-groups of 1024 (8 chunks each);
            # yT DMA issued one group ahead so transfers hide under compute
            # and the group-jt transposes never delay group jt+1's stream ----
            yT_tiles = {}

            def issue_yt(jt):
                t = stgy.tile([128, CCH, 1024], BF16, tag="yT")
                for c in range(CCH):
                    nc.sync.dma_start(out=t[:, c, :],
                                      in_=yT_view[:, c, jt * 1024:(jt + 1) * 1024])
                yT_tiles[jt] = t

            issue_yt(0)
            for jt in range(M // 1024):
                if jt + 1 < M // 1024:
                    issue_yt(jt + 1)
                yT = yT_tiles.pop(jt)
                if jt == 0:
                    emit_x_dma(0)
                    emit_x_dma(1)   # on sync, lands mid-group-0

                # kT halves + ST slices 0,1 + braided AV(0,1)
                for half in range(2):
                    p = proj_half(wk_sb, yT, half)
                    off = jt * 1024 + half * 512
                    drain(kT_sb[:, off:off + 512], p)
                    if jt == 0 and half == 0:
                        q_half(0)
                        q_half(1)
                    if jt == 1 and half == 0:
                        q_half(2)
                    if jt == 1 and half == 1:
                        q_half(3)
                    # AV braid between proj and ST: these matmuls depend only
                    # on last group's exps, filling the kT-drain latency
                    if jt > 0:
                        base = 8 * (jt - 1) + 4 * half
                        for c in range(base, base + 4):
                            emit_av_chunk(0, c)
                            emit_av_chunk(1, c)
                    for cc in (4 * jt + 2 * half, 4 * jt + 2 * half + 1):
                        emit_st_tile(0, cc)
                        emit_st_tile(1, cc)

                # vT halves -> v via DMA XBAR transpose + catch-up STs
                vstage = stgv.tile([128, 1024], BF16, tag="vT")
                for half in range(2):
                    p = proj_half(wv_sb, yT, half)
                    drain(vstage[:, half * 512:(half + 1) * 512], p)
                    if half == 0:
                        for cc in s2_cc.get(jt, []):
                            emit_st_tile(2, cc)
                    else:
                        for cc in s3_cc.get(jt, []):
                            emit_st_tile(3, cc)
                    nc.sync.dma_start_transpose(
                        v_sb[:, 8 * jt + 4 * half:8 * jt + 4 * half + 4, :],
                        vstage[:, half * 512:(half + 1) * 512])

            # ---- tail: last group's AV(0,1), then AV(2), AV(3) ----
            for c in range(24, 32):
                emit_av_chunk(0, c)
                emit_av_chunk(1, c)
            emit_acc_out(0)
            emit_acc_out(1)
            emit_out(0)
            emit_out(1)

            av_tiles[2] = stps.tile([128, 512], F32, tag="st", name="av2")
            av_tiles[3] = stps.tile([128, 512], F32, tag="st", name="av3")
            emit_acc_out(2)
            emit_acc_out(3)
            for c in range(16, 32):
                emit_av_chunk(2, c)
                if c >= 24:
                    emit_av_chunk(3, c)
            emit_out(2)
            emit_out(3)

    nc.compile()
    return nc


def _get_nc():
    if "nc" not in _CACHE:
        _CACHE["nc"] = build_nc()
    return _CACHE["nc"]


def _make_masks(h):
    m = np.zeros((128, 256), dtype=np.float32)
    sl = np.tril(np.ones((128, 128), dtype=np.float32), k=-1)
    if h == 0:
        m[:, 0:128] = sl
        m[:, 128:256] = 1.0
    else:
        m[:, 0:128] = 0.0
        m[:, 128:256] = sl
    return m.astype(NPF16)


def kernel(x, y, Wq, Wk, Wv):
    nc = _get_nc()
    xb = x.astype(NPBF16)
    yb = y.astype(NPBF16)
    wqb = Wq.astype(NPBF16)
    wkb = Wk.astype(NPBF16)
    wvb = Wv.astype(NPBF16)

    in_maps = []
    yT = {b: np.ascontiguousarray(yb[b].T) for b in range(B)}
    for core in range(NCORES):
        b, h = divmod(core, 2)
        xs = xb[b].reshape(N // 128, 128, C)[h::2].reshape(NLOC, C)
        in_maps.append({
            "xsT": np.ascontiguousarray(xs.T),
            "yT": yT[b],
            "wq": wqb, "wk": wkb, "wv": wvb,
            "maskAB": _make_masks(h),
        })

    if TRACE:
        import tempfile
        tdir = tempfile.mkdtemp(prefix="attn_trace_")
        _CACHE["trace_dir"] = tdir
        res = run_bass_kernel_spmd(nc, in_maps, list(range(NCORES)),
                                   trace=True, tmpdir=tdir)
        _CACHE["exec_time_ns"] = res.exec_time_ns
    else:
        res = run_bass_kernel_spmd(nc, in_maps, list(range(NCORES)))

    out = np.empty((B, N, D), dtype=np.float32)
    for core in range(NCORES):
        b, h = divmod(core, 2)
        num = res.results[core]["numT"]                       # [4,128,512]
        accv = res.results[core]["accout"].astype(np.float32)  # [4,128,1024]
        for s in range(NSL):
            Z = accv[s].reshape(128, 2, 512).sum(axis=(0, 1))  # [512]
            o = (num[s] / Z[None, :]).T.reshape(4, 128, D)     # [u, r, d]
            for u in range(4):
                T = 2 * (4 * s + u) + h
                out[b, T * 128:(T + 1) * 128] = o[u]
    return out
